# revision 16
# baseline (speedup 1.0000x reference)
"""2-layer GAT (PyG GATConv semantics) on 8 Trainium2 NeuronCores via Bass/Tile.

Contract: kernel(**inputs) takes the FULL inputs of reference.setup_inputs()
and returns the FULL [16, 4096, 128] float32 output.

Strategy (dst-node sharding, one SPMD program, "one-dst-per-partition" grid):
- Core c owns dst nodes [c*N/8, (c+1)*N/8). Host packs each 128-dst block's
  edges into a [128 x T*KP] slot grid where each PARTITION row holds slots of
  exactly ONE dst (KP slots per row, ceil(deg/KP) rows per dst, T tiles of
  128 rows). This makes the routing matrix per tile a [128,128] one-hot of
  pdloc[p] (one is_equal per tile, not per 128-edge chunk), and a_dst
  selection one transpose+matmul per tile.
- Tables are bf16 with a FAKE row at index N whose a_src = -60000, so padded
  slots get w = exp(leakyrelu(-inf)) = 0 with no masks.
- Phase A (replicated): t1[n] = [h1 | a_src1 | a_dst1] bf16 from a single
  xT @ [W1|Wsrc1|Wdst1] bf16 matmul; t1 rows are ROTATED per core so own
  shard is rows [0, NSH).
- Edge phase (per block): T*KP indirect gathers ([128,1]-offset form, the
  only shape real HW lowers correctly) fill one [128, T*KP, W] tile, ONE
  is_equal builds all T routing matrices, logit/exp/value-weighting are
  single batched ops, and T*KP PSUM-accumulated matmul pairs compute
  acc_v[d] += M_t^T @ (h[src]*w) and acc_s[d] += M_t^T @ w.
- Layer-2 linearity: out[d] = (sum alpha*g[src]) @ W2 with g = elu(y)+1, so
  the aggregation is 64-wide; W2 is applied once per dst block in the L2
  epilogue, a_src2 = g.(W2 a2s) folds the same way, and since sum(alpha)=1
  the elu/g shift folds into bias and logit constants.
- L2 table t2 (AllGathered, slice-major physical layout) rows are
  [g | a_src2' | a_dst2'] bf16 = 132B.
"""

import os
import sys

import numpy as np

if "/opt/trn_rl_repo" not in sys.path:
    sys.path.insert(0, "/opt/trn_rl_repo")

import concourse.bass as bass
import concourse.bacc as bacc
import concourse.mybir as mybir
import concourse.tile as tile

F32 = mybir.dt.float32
BF16 = mybir.dt.bfloat16
I32 = mybir.dt.int32
AOP = mybir.AluOpType
ACT = mybir.ActivationFunctionType

NEG_SLOPE = 0.2
NCORES = 8
BLK = 128
KP = int(os.environ.get("KKP", "8"))   # slots per partition-row
NEG_BIG = -60000.0

T1W = 80   # t1: [0:64] h1, [64:72] asrc1, [72:80] adst1 (gathers read 0:72)
T2W = 66   # t2: [0:64] g,  [64] asrc2',  [65] adst2'    (gathers read 0:65)


class Cfg:
    def __init__(self, n_nodes, d_in, h1, c1, d2, t_tiles, nslice):
        self.N = n_nodes
        self.D = d_in
        self.H1 = h1
        self.C1 = c1
        self.D1 = h1 * c1
        self.D2 = d2
        self.T = t_tiles
        self.TK = t_tiles * KP
        self.NSH = n_nodes // NCORES
        self.NBLK = self.NSH // BLK
        self.NSLICE = nslice
        assert self.NSH % BLK == 0 and self.NBLK % nslice == 0


# ---------------------------------------------------------------------------
# host-side edge schedule
# ---------------------------------------------------------------------------
def _edge_schedule(src, dst, n_nodes):
    """src/dst int64 (self loops included). One-dst-per-partition grid:
    esrc [NCORES, NBLK, 128, T*KP] (node id, FAKE=n_nodes for padding),
    pdloc [NCORES, NBLK, 128, T] (local dst in [0,128) or -1)."""
    nsh = n_nodes // NCORES
    nblk = nsh // BLK
    order = np.argsort(dst, kind="stable")
    src = src[order]
    dst = dst[order]
    ne = len(dst)

    deg = np.bincount(dst, minlength=n_nodes)
    starts = np.zeros(n_nodes + 1, dtype=np.int64)
    np.cumsum(deg, out=starts[1:])
    k_in_dst = np.arange(ne, dtype=np.int64) - starts[dst]

    rows_per_dst = (deg + KP - 1) // KP
    rowstart = np.zeros(n_nodes, dtype=np.int64)
    np.cumsum(rows_per_dst[:-1], out=rowstart[1:])
    first_dst_of_blk = (np.arange(n_nodes) // BLK) * BLK
    row_in_blk = rowstart - rowstart[first_dst_of_blk]

    rows_per_blk = (
        row_in_blk[BLK - 1 :: BLK] + rows_per_dst[BLK - 1 :: BLK]
    )
    t_tiles = int((int(rows_per_blk.max()) + 127) // 128)
    tk = t_tiles * KP

    r = row_in_blk[dst] + k_in_dst // KP          # row in [0, 128*T)
    tt = r // 128
    pp = r % 128
    jj = k_in_dst % KP
    col = tt * KP + jj
    g = dst // BLK
    cc = g // nblk
    bb = g % nblk

    esrc = np.full((NCORES, nblk, 128, tk), n_nodes, dtype=np.int64)
    esrc[cc, bb, pp, col] = src
    pdl = np.full((NCORES, nblk, 128, t_tiles), -1.0, dtype=np.float32)
    pdl[cc, bb, pp, tt] = (dst % BLK).astype(np.float32)
    return t_tiles, esrc, pdl


def _t2_phys(cfg):
    """node id -> physical t2 row (slice-major: slice, rank, local); fake->N."""
    N, NSH, NSLICE = cfg.N, cfg.NSH, cfg.NSLICE
    sl = NSH // NSLICE
    node = np.arange(N + 1, dtype=np.int64)
    r = node // NSH
    loc = node % NSH
    s = loc // sl
    phys = (s * (sl * NCORES) + r * sl + (loc % sl)).astype(np.int64)
    phys[N] = N
    return phys


# ---------------------------------------------------------------------------
# device program
# ---------------------------------------------------------------------------
def _ap(t, dims, offset=0):
    """SBUF/PSUM tile AP: keep partition dim, replace free dims.
    dims = [[stride, size], ...] in elements."""
    a = t[:]
    return bass.AP(a.tensor, a.offset + offset, [list(a.ap[0])] + [list(d) for d in dims])


def build_program(cfg, c2_const, phases="abgc"):
    N, D, H1, D1, D2 = cfg.N, cfg.D, cfg.H1, cfg.D1, cfg.D2
    NSH, NBLK, NSLICE, T, TK = cfg.NSH, cfg.NBLK, cfg.NSLICE, cfg.T, cfg.TK

    nc = bacc.Bacc("TRN2", target_bir_lowering=False, debug=False, num_devices=NCORES)

    xt = nc.dram_tensor("xt", [D, N], BF16, kind="ExternalInput")
    wpack1 = nc.dram_tensor("wpack1", [D, T1W], BF16, kind="ExternalInput")
    w2 = nc.dram_tensor("w2", [D1, D2], BF16, kind="ExternalInput")
    w2a2s = nc.dram_tensor("w2a2s", [128, D1], F32, kind="ExternalInput")
    w2a2d = nc.dram_tensor("w2a2d", [128, D1], F32, kind="ExternalInput")
    b1r = nc.dram_tensor("b1r", [128, D1], F32, kind="ExternalInput")
    b2effr = nc.dram_tensor("b2effr", [128, D2], F32, kind="ExternalInput")
    iota = nc.dram_tensor("iota", [128, 128], BF16, kind="ExternalInput")
    identd = nc.dram_tensor("identd", [128, 128], BF16, kind="ExternalInput")
    fr1 = nc.dram_tensor("fr1", [1, T1W], BF16, kind="ExternalInput")
    fr2 = nc.dram_tensor("fr2", [1, T2W], BF16, kind="ExternalInput")
    esrc1 = nc.dram_tensor("esrc1", [NBLK, 128, TK], I32, kind="ExternalInput")
    esrc2 = nc.dram_tensor("esrc2", [NBLK, 128, TK], I32, kind="ExternalInput")
    pdloc = nc.dram_tensor("pdloc", [NBLK, 128, T], BF16, kind="ExternalInput")
    out = nc.dram_tensor("out", [NSH, D2], F32, kind="ExternalOutput")

    t1 = nc.dram_tensor("t1", [N + 1, T1W], BF16, kind="Internal")
    t2s = nc.dram_tensor("t2s", [NSH, T2W], BF16, kind="Internal")
    t2 = nc.dram_tensor("t2", [N + 1, T2W], BF16, kind="Internal", addr_space="Shared")

    with tile.TileContext(nc) as tc:
        with tc.tile_pool(name="const", bufs=1) as cp:
            con = {}
            for name, hndl, dt_ in [
                ("wpack1", wpack1, BF16), ("w2", w2, BF16),
                ("w2a2s", w2a2s, F32), ("w2a2d", w2a2d, F32),
                ("b1r", b1r, F32), ("b2effr", b2effr, F32),
                ("iota", iota, BF16), ("ident", identd, BF16),
                ("fr1", fr1, BF16), ("fr2", fr2, BF16),
            ]:
                t = cp.tile(list(hndl.shape), dt_, tag=name)
                nc.sync.dma_start(out=t[:], in_=hndl[:])
                con[name] = t

            if "a" in phases:
                _phase_a(nc, tc, cfg, xt, con, t1)
            nc.sync.dma_start(out=t1[N : N + 1, :], in_=con["fr1"][:])
            nc.sync.dma_start(out=t2[N : N + 1, :], in_=con["fr2"][:])
            if "b" in phases:
                _edge_phase(nc, tc, cfg, 1, t1, t2s, t2 if "g" in phases else None,
                            esrc1, pdloc, con, 0.0, None)
            if "c" in phases:
                _edge_phase(nc, tc, cfg, 2, t2, t2s, None, esrc2, pdloc, con,
                            -c2_const, out)

    nc.compile()
    return nc


def _phase_a(nc, tc, cfg, xt, con, t1):
    N = cfg.N
    ntile = N // 128
    group = 8 if ntile % 8 == 0 else 1
    with (
        tc.tile_pool(name="pa_in", bufs=3) as pin,
        tc.tile_pool(name="pa_ps", bufs=4, space="PSUM") as pps,
        tc.tile_pool(name="pa_st", bufs=3) as pst,
    ):
        for mt in range(ntile // group):
            xt_t = pin.tile([cfg.D, 128 * group], BF16, tag="xt")
            nc.sync.dma_start(
                out=xt_t[:], in_=xt[:, mt * 128 * group : (mt + 1) * 128 * group]
            )
            stg = pst.tile([128, group * T1W], BF16, tag="stg")
            for s in range(group):
                ps = pps.tile([128, T1W], F32, tag="ps")
                nc.tensor.matmul(
                    out=ps[:], lhsT=xt_t[:, s * 128 : (s + 1) * 128],
                    rhs=con["wpack1"][:], start=True, stop=True,
                )
                nc.scalar.copy(out=stg[:, s * T1W : (s + 1) * T1W], in_=ps[:])
            dst_ap = bass.AP(
                t1[:].tensor,
                mt * 128 * group * T1W,
                [[T1W, 128], [128 * T1W, group], [1, T1W]],
            )
            nc.sync.dma_start(
                out=dst_ap, in_=stg[:].rearrange("p (s w) -> p s w", w=T1W)
            )


def _edge_phase(nc, tc, cfg, layer, table, t2s, t2, esrc, pdloc, con, logit_c, out):
    """layer 1: table=t1 (rotated, own shard = rows [0,NSH)), writes t2s and
    (sliced) AllGathers into t2.  layer 2: table=t2, writes out."""
    NBLK, NSLICE, T, TK = cfg.NBLK, cfg.NSLICE, cfg.T, cfg.TK
    BPS = NBLK // NSLICE
    SLN = cfg.NSH // NSLICE
    H1, C1, D1, D2 = cfg.H1, cfg.C1, cfg.D1, cfg.D2
    if layer == 1:
        D, H, TW, acol = D1, cfg.H1, T1W, 72
        awin_src = table
    else:
        D, H, TW, acol = D1, 1, T2W, 65
        awin_src = t2s
    GW = D + H          # gathered row prefix width (values + a_src)
    RC = D + H          # rhs/acc width: [v | w]
    L = f"e{layer}"
    with (
        tc.tile_pool(name=L + "_ix", bufs=3) as pix,
        tc.tile_pool(name=L + "_g", bufs=4) as pg,
        tc.tile_pool(name=L + "_m", bufs=3) as pm,
        tc.tile_pool(name=L + "_r", bufs=3) as pr,
        tc.tile_pool(name=L + "_acc", bufs=2, space="PSUM") as pacc,
        tc.tile_pool(name=L + "_acs", bufs=2, space="PSUM") as pacs,
        tc.tile_pool(name=L + "_mtp", bufs=1, space="PSUM") as pmtp,
        tc.tile_pool(name=L + "_ad", bufs=1, space="PSUM") as pad,
        tc.tile_pool(name=L + "_ep", bufs=2) as pep,
        tc.tile_pool(name=L + "_epp", bufs=1, space="PSUM") as pepp,
    ):
        for b in range(NBLK):
            src_t = pix.tile([128, TK], I32, tag="src")
            nc.sync.dma_start(out=src_t[:], in_=esrc[b])
            pdl_t = pix.tile([128, T], BF16, tag="pdl")
            nc.sync.dma_start(out=pdl_t[:], in_=pdloc[b])
            adw = pix.tile([128, H], BF16, tag="adw")
            nc.sync.dma_start(
                out=adw[:], in_=awin_src[b * BLK : (b + 1) * BLK, acol : acol + H]
            )

            # gather slot rows: [128, TK, GW] <- table[src]; one indirect DMA
            # per slot column ([128,1] offsets is the only HW-proven shape)
            gat = pg.tile([128, TK * GW], BF16, tag="gat")
            for c in range(TK):
                nc.gpsimd.indirect_dma_start(
                    out=_ap(gat, [[1, GW]], offset=c * GW),
                    out_offset=None, in_=table[:],
                    in_offset=bass.IndirectOffsetOnAxis(
                        ap=src_t[:, c : c + 1], axis=0),
                )

            # all T routing matrices in one is_equal
            m_all = pm.tile([128, T * 128], BF16, tag="m")
            nc.vector.tensor_tensor(
                out=_ap(m_all, [[128, T], [1, 128]]),
                in0=_ap(pdl_t, [[1, T], [0, 128]]),
                in1=_ap(con["iota"], [[0, T], [1, 128]]),
                op=AOP.is_equal,
            )

            # a_dst per partition-row, replicated over KP slots:
            # ad_all[p, (t,j,h)] = adw[pd_t[p], h] via M^T selection
            adw_rep = pix.tile([128, KP * H], BF16, tag="adwr")
            nc.vector.tensor_copy(
                out=_ap(adw_rep, [[H, KP], [1, H]]),
                in_=_ap(adw, [[0, KP], [1, H]]),
            )
            ad_all = pad.tile([128, TK * H], F32, tag="ad")
            for t in range(T):
                mtp = pmtp.tile([128, 128], BF16, tag="mtp")
                nc.tensor.transpose(
                    out=mtp[:], in_=m_all[:, t * 128 : (t + 1) * 128],
                    identity=con["ident"][:],
                )
                mts = pm.tile([128, 128], BF16, tag="mts")
                nc.scalar.copy(out=mts[:], in_=mtp[:])
                nc.tensor.matmul(
                    out=ad_all[:, t * KP * H : (t + 1) * KP * H], lhsT=mts[:],
                    rhs=adw_rep[:], start=True, stop=True,
                )

            # logits -> w (batched over the whole block; all APs <= 3D)
            lg = pr.tile([128, TK * H], F32, tag="lg")
            nc.vector.scalar_tensor_tensor(
                out=lg[:],
                in0=ad_all[:],
                scalar=float(logit_c),
                in1=_ap(gat, [[GW, TK], [1, H]], offset=D),
                op0=AOP.add, op1=AOP.add,
            )
            lr = pr.tile([128, TK * H], F32, tag="lr")
            nc.vector.scalar_tensor_tensor(
                out=lr[:], in0=lg[:], scalar=NEG_SLOPE, in1=lg[:],
                op0=AOP.mult, op1=AOP.max,
            )
            # w expanded per value column: w_exp[p, (slot, h, c)] = w[p, slot, h]
            CC = D // H
            w_exp = pr.tile([128, TK * D], BF16, tag="wx")
            nc.scalar.activation(
                out=_ap(w_exp, [[CC, TK * H], [1, CC]]),
                in_=_ap(lr, [[1, TK * H], [0, CC]]),
                func=ACT.Exp,
            )
            v_t = pr.tile([128, TK * D], BF16, tag="v")
            nc.vector.tensor_tensor(
                out=v_t[:],
                in0=_ap(gat, [[GW, TK], [1, D]]),
                in1=w_exp[:],
                op=AOP.mult,
            )

            # routed accumulation: acc_v += M^T v, acc_s += M^T w
            acc_v = pacc.tile([128, D], F32, tag="accv")
            acc_s = pacs.tile([128, H], F32, tag="accs")
            for t in range(T):
                lhsT = m_all[:, t * 128 : (t + 1) * 128]
                for j in range(KP):
                    c = t * KP + j
                    first = c == 0
                    last = c == TK - 1
                    nc.tensor.matmul(
                        out=acc_v[:], lhsT=lhsT,
                        rhs=_ap(v_t, [[1, D]], offset=c * D),
                        start=first, stop=last,
                    )
                    nc.tensor.matmul(
                        out=acc_s[:], lhsT=lhsT,
                        rhs=_ap(w_exp, [[CC, H]], offset=c * D),
                        start=first, stop=last,
                    )

            # ---------------- block epilogue ------------------------------
            sinv = pep.tile([128, H], F32, tag="sinv")
            nc.vector.reciprocal(out=sinv[:], in_=acc_s[:])

            if layer == 2:
                o1 = pep.tile([128, D1], F32, tag="o1")
                nc.scalar.activation(
                    out=o1[:], in_=acc_v[:], func=ACT.Copy, scale=sinv[:, 0:1]
                )
                o1b = pep.tile([128, D1], BF16, tag="o1b")
                nc.vector.tensor_copy(out=o1b[:], in_=o1[:])
                atp = pepp.tile([D1, 128], BF16, tag="atp")
                nc.tensor.transpose(out=atp[:], in_=o1b[:], identity=con["ident"][:])
                ats = pep.tile([D1, 128], BF16, tag="ats")
                nc.scalar.copy(out=ats[:], in_=atp[:])
                ops = pepp.tile([128, D2], F32, tag="ops")
                nc.tensor.matmul(
                    out=ops[:], lhsT=ats[:], rhs=con["w2"][:], start=True, stop=True
                )
                o2 = pep.tile([128, D2], F32, tag="o2")
                nc.vector.tensor_add(out=o2[:], in0=ops[:], in1=con["b2effr"][:])
                nc.sync.dma_start(out=out[b * BLK : (b + 1) * BLK, :], in_=o2[:])
                continue

            # layer 1: y = acc_v/s + b1; t2s row = [elu(y)|asrc'|adst']
            y = pep.tile([128, D1], F32, tag="y")
            nc.vector.tensor_tensor(
                out=_ap(y, [[C1, H1], [1, C1]]),
                in0=_ap(acc_v, [[C1, H1], [1, C1]]),
                in1=_ap(sinv, [[1, H1], [0, C1]]),
                op=AOP.mult,
            )
            nc.vector.tensor_add(out=y[:], in0=y[:], in1=con["b1r"][:])
            tmin = pep.tile([128, D1], F32, tag="tmin")
            nc.vector.tensor_scalar_min(out=tmin[:], in0=y[:], scalar1=0.0)
            e_t = pep.tile([128, D1], F32, tag="e")
            nc.scalar.activation(out=e_t[:], in_=tmin[:], func=ACT.Exp)
            g_t = pep.tile([128, D1], F32, tag="g")
            nc.vector.scalar_tensor_tensor(
                out=g_t[:], in0=y[:], scalar=0.0, in1=e_t[:],
                op0=AOP.max, op1=AOP.add,
            )
            # center: store elu = g - 1 (avoids bf16 cancellation in A@W2)
            eluf = pep.tile([128, D1], F32, tag="eluf")
            nc.vector.tensor_scalar_add(out=eluf[:], in0=g_t[:], scalar1=-1.0)
            stg2 = pep.tile([128, T2W], BF16, tag="stg2")
            nc.vector.tensor_copy(out=stg2[:, 0:D1], in_=eluf[:])
            scr = pep.tile([128, D1], F32, tag="scr")
            nc.vector.tensor_tensor(
                out=scr[:], in0=eluf[:], in1=con["w2a2s"][:], op=AOP.mult
            )
            with nc.allow_low_precision(reason="bf16 logit terms, tol 2e-2"):
                nc.vector.reduce_sum(
                    out=stg2[:, D1 : D1 + 1], in_=scr[:], axis=mybir.AxisListType.X
                )
            nc.vector.tensor_tensor(
                out=scr[:], in0=eluf[:], in1=con["w2a2d"][:], op=AOP.mult
            )
            with nc.allow_low_precision(reason="bf16 logit terms, tol 2e-2"):
                nc.vector.reduce_sum(
                    out=stg2[:, D1 + 1 : D1 + 2], in_=scr[:], axis=mybir.AxisListType.X
                )
            nc.sync.dma_start(out=t2s[b * BLK : (b + 1) * BLK, :], in_=stg2[:])

            if t2 is not None and (b + 1) % BPS == 0:
                s = (b + 1) // BPS - 1
                nc.gpsimd.collective_compute(
                    "AllGather",
                    AOP.bypass,
                    replica_groups=[list(range(NCORES))],
                    ins=[t2s[s * SLN : (s + 1) * SLN, :]],
                    outs=[t2[s * SLN * NCORES : (s + 1) * SLN * NCORES, :]],
                )


# ---------------------------------------------------------------------------
# host glue
# ---------------------------------------------------------------------------
def prepare(x, seq, edges, W1, att_src1, att_dst1, b1, W2, att_src2,
            att_dst2, b2, nslice=4):
    nb, ncn, d = x.shape
    N = nb * ncn
    H1, C1 = att_src1.shape
    D1 = H1 * C1
    D2 = W2.shape[1]

    xf = (np.asarray(x, np.float32).reshape(N, d)
          * np.asarray(seq, np.float32).reshape(N, 1))
    src = np.concatenate([np.asarray(edges[0], np.int64), np.arange(N, dtype=np.int64)])
    dst = np.concatenate([np.asarray(edges[1], np.int64), np.arange(N, dtype=np.int64)])
    t_tiles, esrc_g, pdl = _edge_schedule(src, dst, N)
    cfg = Cfg(N, d, H1, C1, D2, t_tiles, nslice)

    w1 = np.asarray(W1, np.float32)
    wsrc = np.einsum("khc,hc->kh", w1.reshape(d, H1, C1), np.asarray(att_src1, np.float32))
    wdst = np.einsum("khc,hc->kh", w1.reshape(d, H1, C1), np.asarray(att_dst1, np.float32))
    wpack1 = np.concatenate([w1, wsrc, wdst], axis=1).astype(np.float32)

    w2a = np.asarray(W2, np.float32)
    a2s = np.asarray(att_src2, np.float32).reshape(-1)
    a2d = np.asarray(att_dst2, np.float32).reshape(-1)
    # t2 stores centered elu values, so no colsum / logit-constant folds
    c2_const = 0.0
    b2eff = np.asarray(b2, np.float32)
    w2s = w2a @ a2s    # [D1]
    w2d = w2a @ a2d

    def bf(a):
        import jax.numpy as jnp
        return np.asarray(jnp.asarray(a, jnp.bfloat16))

    w2a2s = np.tile(w2s[None, :], (128, 1)).astype(np.float32)
    w2a2d = np.tile(w2d[None, :], (128, 1)).astype(np.float32)
    b1r = np.tile(np.asarray(b1, np.float32)[None, :], (128, 1)).astype(np.float32)
    b2effr = np.tile(b2eff[None, :], (128, 1)).astype(np.float32)
    iota = np.tile(np.arange(128, dtype=np.float32)[None, :], (128, 1))
    ident = np.eye(128, dtype=np.float32)
    fr1 = np.zeros((1, T1W), np.float32)
    fr1[0, D1 : D1 + H1] = NEG_BIG
    fr2 = np.zeros((1, T2W), np.float32)
    fr2[0, D1] = NEG_BIG

    phys = _t2_phys(cfg)
    in_maps = []
    for c in range(NCORES):
        rot = (np.arange(N, dtype=np.int64) + c * cfg.NSH) % N
        xt_c = bf(np.ascontiguousarray(xf[rot].T))
        e1 = np.where(
            esrc_g[c] == N, N, (esrc_g[c] - c * cfg.NSH) % N
        ).astype(np.int32)
        e2 = phys[esrc_g[c]].astype(np.int32)
        in_maps.append(
            {
                "xt": xt_c,
                "wpack1": bf(wpack1),
                "w2": bf(w2a),
                "w2a2s": w2a2s,
                "w2a2d": w2a2d,
                "b1r": b1r,
                "b2effr": b2effr,
                "iota": bf(iota),
                "identd": bf(ident),
                "fr1": bf(fr1),
                "fr2": bf(fr2),
                "esrc1": e1,
                "esrc2": e2,
                "pdloc": bf(pdl[c]),
            }
        )
    return cfg, c2_const, in_maps


_CACHE = {}
LAST_RESULT = None


def kernel(**inputs) -> np.ndarray:
    from concourse.bass_utils import run_bass_kernel_spmd

    global LAST_RESULT
    x = np.asarray(inputs["x"])
    nb, ncn, d = x.shape
    cfg, c2_const, in_maps = prepare(**{k: inputs[k] for k in (
        "x", "seq", "edges", "W1", "att_src1", "att_dst1", "b1",
        "W2", "att_src2", "att_dst2", "b2")})

    key = (cfg.N, cfg.D, cfg.H1, cfg.C1, cfg.D2, cfg.T, KP, cfg.NSLICE,
           round(c2_const, 10))
    if key not in _CACHE:
        _CACHE.clear()
        _CACHE[key] = build_program(cfg, c2_const)
    nc = _CACHE[key]

    res = run_bass_kernel_spmd(nc, in_maps, core_ids=list(range(NCORES)), trace=False)
    LAST_RESULT = res
    shards = [res.results[c]["out"] for c in range(NCORES)]
    full = np.concatenate(shards, axis=0)
    return full.reshape(nb, ncn, d).astype(np.float32)
